# revision 9
# baseline (speedup 1.0000x reference)
"""Bass/Trainium2 kernel for nn_D2A_12086037971054 (sparse_attention).

Reference computation:
    Att[i,j] = MLP(DistM[i,j])           (per-scalar 1->64(tanh)->1 MLP)
    Att     *= (1 - I)                   (off-diagonal mask)
    Att     /= (Att.sum(axis=0) + EPS)   (column normalization)
    out      = x + x @ Att               ([16,4096,1024] @ [1024,1024])
    penalty  = scalar from 32 keypoints through the same MLP

Strategy (8 NeuronCores, data-parallel over batch per the sharding hint):
  - Att is tiny ([1024,1024], ~0.1% of total FLOPs) -> computed on host in
    fp32, residual folded in as M = I + Att so the device does one dense
    matmul out = x @ M (no separate residual add / second read of x).
  - x is viewed as [65536, 1024], split into 8 row-shards of 8192 rows, each
    shard transposed on host so the contraction dim lands on SBUF partitions
    with fully contiguous DMA.
  - Precision trick: fp32 matmul on the PE costs 4 cycles/row (two half-rate
    passes). Instead we split each operand into fp16 hi + fp16 lo
    (11+11 mantissa bits) and accumulate three fp16 matmuls in PSUM fp32:
        x@M ~= xh@Mh + xh@Ml + xl@Mh        (dropped xl@Ml ~ 2^-22 rel)
    3 cycles/row, HW-measured accuracy identical to the PE's native fp32
    path (~2e-7 max rel err), and the same DMA bytes as fp32.
  - penalty depends only on the MLP weights; computed host-side (replicated).
"""

import numpy as np

import concourse.mybir as mybir
from concourse import bacc, tile
from concourse.bass_utils import run_bass_kernel_spmd

N_CORES = 8
B, S, F, H = 16, 4096, 1024, 64
ROWS = B * S  # 65536
RPC = ROWS // N_CORES  # rows per core: 8192
PENALTY_RATE = 0.1
NUM_KPTS = 32
EPS = 1e-7

P = 128
KO = F // P  # 8 contraction tiles
R_CHUNK = 512  # rows of x streamed per DMA panel
N_HALF = 512  # PSUM bank width (fp32)

_FP32 = mybir.dt.float32
_FP16 = mybir.dt.float16

LAST_RESULTS = None  # BassKernelResults of the most recent run (for test.py)


def _dedupe_ldweights(nc):
    """Drop PE InstLdweights that reload the exact weights AP already loaded
    by the immediately preceding (PE-stream) InstLdweights.

    tile_legalize splits every matmul into Ldweights + Matmult(ldweights=False);
    consecutive matmuls that share a stationary tile then carry redundant
    loads, which cost unhidden PE issue time at short (N=512 fp16, ~213ns)
    matmuls. Only sync-free loads are removed, so semaphore accounting is
    untouched; the PE weight state persists across matmuls, making the drop
    semantics-preserving."""
    import json as _json

    removed = 0
    for func in nc.m.functions:
        for block in func.blocks:
            new = []
            prev_sig = None
            for inst in block.instructions:
                if inst.engine == mybir.EngineType.PE:
                    if isinstance(inst, mybir.InstLdweights):
                        sig = str(inst.ins[0])
                        d = _json.loads(mybir.instruction_to_pretty_json_string(inst))
                        si = d.get("sync_info") or {}
                        clean = not si.get("on_wait") and not si.get("on_update")
                        if clean and sig == prev_sig:
                            removed += 1
                            continue
                        prev_sig = sig
                    elif isinstance(inst, mybir.InstMatmult):
                        if getattr(inst, "ldweights", None) is not False:
                            prev_sig = None  # self-loading matmul changes state
                    else:
                        prev_sig = None  # any other PE inst: be conservative
                new.append(inst)
            block.instructions = new
    return removed


def _build_nc_fp16x2(n_loops=1):
    """out = xh.T@Mh + xh.T@Ml + xl.T@Mh, all fp16 operands, fp32 PSUM.

    Matmuls are ordered so each stationary tile (xh_k / xl_k) serves the
    two PSUM half-banks back-to-back, letting _dedupe_ldweights drop 2/3 of
    the weight loads.

    n_loops > 1 repeats the whole compute body inside the NEFF; used only
    for differential HW timing ((t_N - t_1) / (N - 1))."""
    nc = bacc.Bacc(None, target_bir_lowering=False)
    xhT = nc.dram_tensor("xhT", [F, RPC], _FP16, kind="ExternalInput")
    xlT = nc.dram_tensor("xlT", [F, RPC], _FP16, kind="ExternalInput")
    mh = nc.dram_tensor("mh", [F, F], _FP16, kind="ExternalInput")
    ml = nc.dram_tensor("ml", [F, F], _FP16, kind="ExternalInput")
    out = nc.dram_tensor("out", [RPC, F], _FP32, kind="ExternalOutput")

    with tile.TileContext(nc) as tc:
        with (
            tc.tile_pool(name="mpool", bufs=1) as mpool,
            tc.tile_pool(name="xpool", bufs=4) as xpool,
            tc.tile_pool(name="opool", bufs=4) as opool,
            tc.tile_pool(name="pspool", bufs=2, space="PSUM") as pspool,
        ):
            # M hi/lo resident in SBUF: [128, 8, 1024] fp16 each.
            # DMA routing: x-hi + M-hi on the SP HWDGE ring, x-lo + M-lo on
            # the ACT HWDGE ring, output stores on the gpsimd SWDGE path —
            # three independent queues so panel loads never sit behind
            # output stores in a FIFO (measured ~100us of exposed stall when
            # everything shared the SP ring).
            mh_sb = mpool.tile([P, KO, F], _FP16, name="mh_sb")
            ml_sb = mpool.tile([P, KO, F], _FP16, name="ml_sb")
            nc.sync.dma_start(mh_sb[:], mh.rearrange("(ko p) n -> p ko n", p=P))
            nc.scalar.dma_start(ml_sb[:], ml.rearrange("(ko p) n -> p ko n", p=P))

            xh3 = xhT.rearrange("(ko p) r -> p ko r", p=P)
            xl3 = xlT.rearrange("(ko p) r -> p ko r", p=P)
            for _it in range(n_loops):
                for rc in range(RPC // R_CHUNK):  # 16 panels
                    xh_t = xpool.tile([P, KO, R_CHUNK], _FP16, name="xh_t")
                    xl_t = xpool.tile([P, KO, R_CHUNK], _FP16, name="xl_t")
                    rsl = slice(rc * R_CHUNK, (rc + 1) * R_CHUNK)
                    nc.sync.dma_start(xh_t[:], xh3[:, :, rsl])
                    nc.scalar.dma_start(xl_t[:], xl3[:, :, rsl])
                    for rs in range(R_CHUNK // P):  # 4 row-tiles per panel
                        ot = opool.tile([P, F], _FP32, name="ot")
                        ps0 = pspool.tile([P, N_HALF], _FP32, tag="ps0")
                        ps1 = pspool.tile([P, N_HALF], _FP32, tag="ps1")
                        for ko in range(KO):
                            xh_k = xh_t[:, ko, rs * P : (rs + 1) * P]
                            xl_k = xl_t[:, ko, rs * P : (rs + 1) * P]
                            st = ko == 0
                            sp = ko == KO - 1
                            # xh_k stationary: 4 matmuls; xl_k: 2 matmuls
                            nc.tensor.matmul(ps0[:], xh_k, mh_sb[:, ko, 0:N_HALF], start=st, stop=False)
                            nc.tensor.matmul(ps1[:], xh_k, mh_sb[:, ko, N_HALF:F], start=st, stop=False)
                            nc.tensor.matmul(ps0[:], xh_k, ml_sb[:, ko, 0:N_HALF], start=False, stop=False)
                            nc.tensor.matmul(ps1[:], xh_k, ml_sb[:, ko, N_HALF:F], start=False, stop=False)
                            nc.tensor.matmul(ps0[:], xl_k, mh_sb[:, ko, 0:N_HALF], start=False, stop=sp)
                            nc.tensor.matmul(ps1[:], xl_k, mh_sb[:, ko, N_HALF:F], start=False, stop=sp)
                        nc.vector.tensor_copy(ot[:, 0:N_HALF], ps0[:])
                        nc.vector.tensor_copy(ot[:, N_HALF:F], ps1[:])
                        r0 = rc * R_CHUNK + rs * P
                        nc.gpsimd.dma_start(out[r0 : r0 + P, :], ot[:])
    nc.compile()
    _dedupe_ldweights(nc)
    return nc


def _build_nc(n_loops=1, mm_dtype=_FP32):
    """Plain single-dtype variant (fp32 reference path / experiments)."""
    nc = bacc.Bacc(None, target_bir_lowering=False)
    xT = nc.dram_tensor("xT", [F, RPC], mm_dtype, kind="ExternalInput")
    mm = nc.dram_tensor("mm", [F, F], mm_dtype, kind="ExternalInput")
    out = nc.dram_tensor("out", [RPC, F], _FP32, kind="ExternalOutput")

    with tile.TileContext(nc) as tc:
        with (
            tc.tile_pool(name="mpool", bufs=1) as mpool,
            tc.tile_pool(name="xpool", bufs=3) as xpool,
            tc.tile_pool(name="opool", bufs=3) as opool,
            tc.tile_pool(name="pspool", bufs=2, space="PSUM") as pspool,
        ):
            m_sb = mpool.tile([P, KO, F], mm_dtype)
            nc.sync.dma_start(m_sb[:], mm.rearrange("(ko p) n -> p ko n", p=P))

            xT3 = xT.rearrange("(ko p) r -> p ko r", p=P)
            for _it in range(n_loops):
                for rc in range(RPC // R_CHUNK):  # 16 panels
                    xt = xpool.tile([P, KO, R_CHUNK], mm_dtype)
                    nc.sync.dma_start(
                        xt[:], xT3[:, :, rc * R_CHUNK : (rc + 1) * R_CHUNK]
                    )
                    for rs in range(R_CHUNK // P):  # 4 row-tiles per panel
                        ot = opool.tile([P, F], _FP32)
                        for half in range(2):
                            ps = pspool.tile([P, N_HALF], _FP32, tag=f"ps{half}")
                            n0 = half * N_HALF
                            for ko in range(KO):
                                nc.tensor.matmul(
                                    ps[:],
                                    xt[:, ko, rs * P : (rs + 1) * P],
                                    m_sb[:, ko, n0 : n0 + N_HALF],
                                    start=(ko == 0),
                                    stop=(ko == KO - 1),
                                )
                            nc.vector.tensor_copy(ot[:, n0 : n0 + N_HALF], ps[:])
                        r0 = rc * R_CHUNK + rs * P
                        nc.sync.dma_start(out[r0 : r0 + P, :], ot[:])
    nc.compile()
    return nc


_NC_CACHE = {}


def _get_nc(mode="fp16x2"):
    if mode not in _NC_CACHE:
        if mode == "fp16x2":
            _NC_CACHE[mode] = _build_nc_fp16x2()
        elif mode == "fp32":
            _NC_CACHE[mode] = _build_nc()
        else:
            raise ValueError(mode)
    return _NC_CACHE[mode]


def _mlp(z, W1, b1, W2, b2):
    # (N,1) -> (N,1) fp32, mirrors reference._map_mlp
    h = np.tanh(z @ W1 + b1)
    return h @ W2 + b2


def _host_att_and_penalty(DistM, W1, b1, W2, b2):
    f = DistM.shape[0]
    att = _mlp(DistM.reshape(-1, 1), W1, b1, W2, b2).reshape(f, f)
    att = att * (1.0 - np.eye(f, dtype=att.dtype))
    att = att / (att.sum(axis=0, keepdims=True, dtype=np.float32) + EPS)
    m = att + np.eye(f, dtype=att.dtype)  # fold residual: x + x@Att = x @ (I+Att)

    kpts = np.linspace(0.0, 1.0, NUM_KPTS, dtype=np.float32).reshape(-1, 1)
    kout = _mlp(kpts, W1, b1, W2, b2)[:, 0]
    diff = kout[1:] - kout[:-1]
    penalty = 0.5 * PENALTY_RATE * np.mean(np.abs(diff) + diff, dtype=np.float32)
    return np.ascontiguousarray(m, dtype=np.float32), np.float32(penalty)


def _split16(a):
    hi = a.astype(np.float16)
    lo = (a - hi.astype(np.float32)).astype(np.float16)
    return hi, lo


def kernel(x, DistM, W1, b1, W2, b2, trace=False, trace_cores=None, mode="fp16x2"):
    global LAST_RESULTS
    x = np.asarray(x, dtype=np.float32)
    DistM = np.asarray(DistM, dtype=np.float32)
    W1 = np.asarray(W1, dtype=np.float32)
    b1 = np.asarray(b1, dtype=np.float32)
    W2 = np.asarray(W2, dtype=np.float32)
    b2 = np.asarray(b2, dtype=np.float32)

    m, penalty = _host_att_and_penalty(DistM, W1, b1, W2, b2)

    x2d = x.reshape(ROWS, F)
    if mode == "fp16x2":
        xT = np.ascontiguousarray(x2d.T)  # [F, ROWS]
        xh, xl = _split16(xT)
        mh, ml = _split16(m)
        in_maps = [
            {
                "xhT": xh[:, c * RPC : (c + 1) * RPC],
                "xlT": xl[:, c * RPC : (c + 1) * RPC],
                "mh": mh,
                "ml": ml,
            }
            for c in range(N_CORES)
        ]
    else:
        in_maps = [
            {"xT": x2d[c * RPC : (c + 1) * RPC].T, "mm": m} for c in range(N_CORES)
        ]

    kwargs = {}
    if trace:
        kwargs["trace"] = True
        if trace_cores is not None:
            kwargs["trace_cores"] = trace_cores

    res = run_bass_kernel_spmd(
        _get_nc(mode), in_maps, core_ids=list(range(N_CORES)), **kwargs
    )
    LAST_RESULTS = res

    out = np.concatenate(
        [res.results[c]["out"] for c in range(N_CORES)], axis=0
    ).reshape(B, S, F)
    return out, penalty


# revision 10
# speedup vs baseline: 7.6436x; 7.6436x over previous
"""Bass/Trainium2 kernel for nn_D2A_12086037971054 (sparse_attention).

Reference computation:
    Att[i,j] = MLP(DistM[i,j])           (per-scalar 1->64(tanh)->1 MLP)
    Att     *= (1 - I)                   (off-diagonal mask)
    Att     /= (Att.sum(axis=0) + EPS)   (column normalization)
    out      = x + x @ Att               ([16,4096,1024] @ [1024,1024])
    penalty  = scalar from 32 keypoints through the same MLP

Strategy (8 NeuronCores, data-parallel over batch per the sharding hint):
  - Att is tiny ([1024,1024], ~0.1% of total FLOPs) -> computed on host in
    fp32, residual folded in as M = I + Att so the device does one dense
    matmul out = x @ M (no separate residual add / second read of x).
  - x is viewed as [65536, 1024], split into 8 row-shards of 8192 rows, each
    shard transposed on host so the contraction dim lands on SBUF partitions
    with fully contiguous DMA.
  - Precision trick: fp32 matmul on the PE costs 4 cycles/row (two half-rate
    passes). Instead we split each operand into fp16 hi + fp16 lo
    (11+11 mantissa bits) and accumulate three fp16 matmuls in PSUM fp32:
        x@M ~= xh@Mh + xh@Ml + xl@Mh        (dropped xl@Ml ~ 2^-22 rel)
    3 cycles/row, HW-measured accuracy identical to the PE's native fp32
    path (~2e-7 max rel err), and the same DMA bytes as fp32.
  - penalty depends only on the MLP weights; computed host-side (replicated).
"""

import numpy as np

import concourse.mybir as mybir
from concourse import bacc, tile
from concourse.bass_utils import run_bass_kernel_spmd

N_CORES = 8
B, S, F, H = 16, 4096, 1024, 64
ROWS = B * S  # 65536
RPC = ROWS // N_CORES  # rows per core: 8192
PENALTY_RATE = 0.1
NUM_KPTS = 32
EPS = 1e-7

P = 128
KO = F // P  # 8 contraction tiles
R_CHUNK = 512  # rows of x streamed per DMA panel
N_HALF = 512  # PSUM bank width (fp32)

_FP32 = mybir.dt.float32
_FP16 = mybir.dt.float16

LAST_RESULTS = None  # BassKernelResults of the most recent run (for test.py)


def _dedupe_ldweights(nc):
    """Drop PE InstLdweights that reload the exact weights AP already loaded
    by the immediately preceding (PE-stream) InstLdweights.

    tile_legalize splits every matmul into Ldweights + Matmult(ldweights=False);
    consecutive matmuls that share a stationary tile then carry redundant
    loads, which cost unhidden PE issue time at short (N=512 fp16, ~213ns)
    matmuls. Only sync-free loads are removed, so semaphore accounting is
    untouched; the PE weight state persists across matmuls, making the drop
    semantics-preserving."""
    import json as _json

    removed = 0
    for func in nc.m.functions:
        for block in func.blocks:
            new = []
            prev_sig = None
            for inst in block.instructions:
                if inst.engine == mybir.EngineType.PE:
                    if isinstance(inst, mybir.InstLdweights):
                        sig = str(inst.ins[0])
                        d = _json.loads(mybir.instruction_to_pretty_json_string(inst))
                        si = d.get("sync_info") or {}
                        clean = not si.get("on_wait") and not si.get("on_update")
                        if clean and sig == prev_sig:
                            removed += 1
                            continue
                        prev_sig = sig
                    elif isinstance(inst, mybir.InstMatmult):
                        if getattr(inst, "ldweights", None) is not False:
                            prev_sig = None  # self-loading matmul changes state
                    else:
                        prev_sig = None  # any other PE inst: be conservative
                new.append(inst)
            block.instructions = new
    return removed


def _build_nc_fp16x2(n_loops=1):
    """out = xh.T@Mh + xh.T@Ml + xl.T@Mh, all fp16 operands, fp32 PSUM.

    Matmuls are ordered so each stationary tile (xh_k / xl_k) serves the
    two PSUM half-banks back-to-back, letting _dedupe_ldweights drop 2/3 of
    the weight loads.

    n_loops > 1 repeats the whole compute body inside the NEFF; used only
    for differential HW timing ((t_N - t_1) / (N - 1))."""
    nc = bacc.Bacc(None, target_bir_lowering=False)
    xhT = nc.dram_tensor("xhT", [F, RPC], _FP16, kind="ExternalInput")
    xlT = nc.dram_tensor("xlT", [F, RPC], _FP16, kind="ExternalInput")
    mh = nc.dram_tensor("mh", [F, F], _FP16, kind="ExternalInput")
    ml = nc.dram_tensor("ml", [F, F], _FP16, kind="ExternalInput")
    out = nc.dram_tensor("out", [RPC, F], _FP32, kind="ExternalOutput")

    with tile.TileContext(nc) as tc:
        with (
            tc.tile_pool(name="mpool", bufs=1) as mpool,
            tc.tile_pool(name="xpool", bufs=4) as xpool,
            tc.tile_pool(name="opool", bufs=4) as opool,
            tc.tile_pool(name="pspool", bufs=2, space="PSUM") as pspool,
        ):
            # M hi/lo resident in SBUF: [128, 8, 1024] fp16 each.
            # DMA routing: x-hi + M-hi on the SP HWDGE ring, x-lo + M-lo on
            # the ACT HWDGE ring, output stores on the gpsimd SWDGE path —
            # three independent queues so panel loads never sit behind
            # output stores in a FIFO (measured ~100us of exposed stall when
            # everything shared the SP ring).
            mh_sb = mpool.tile([P, KO, F], _FP16, name="mh_sb")
            ml_sb = mpool.tile([P, KO, F], _FP16, name="ml_sb")
            nc.sync.dma_start(mh_sb[:], mh.rearrange("(ko p) n -> p ko n", p=P))
            nc.scalar.dma_start(ml_sb[:], ml.rearrange("(ko p) n -> p ko n", p=P))

            xh3 = xhT.rearrange("(ko p) r -> p ko r", p=P)
            xl3 = xlT.rearrange("(ko p) r -> p ko r", p=P)
            for _it in range(n_loops):
                for rc in range(RPC // R_CHUNK):  # 16 panels
                    xh_t = xpool.tile([P, KO, R_CHUNK], _FP16, name="xh_t")
                    xl_t = xpool.tile([P, KO, R_CHUNK], _FP16, name="xl_t")
                    rsl = slice(rc * R_CHUNK, (rc + 1) * R_CHUNK)
                    nc.sync.dma_start(xh_t[:], xh3[:, :, rsl])
                    nc.scalar.dma_start(xl_t[:], xl3[:, :, rsl])
                    for rs in range(R_CHUNK // P):  # 4 row-tiles per panel
                        ot = opool.tile([P, F], _FP32, name="ot")
                        ps0 = pspool.tile([P, N_HALF], _FP32, tag="ps0")
                        ps1 = pspool.tile([P, N_HALF], _FP32, tag="ps1")
                        for ko in range(KO):
                            xh_k = xh_t[:, ko, rs * P : (rs + 1) * P]
                            xl_k = xl_t[:, ko, rs * P : (rs + 1) * P]
                            st = ko == 0
                            sp = ko == KO - 1
                            # xh_k stationary: 4 matmuls; xl_k: 2 matmuls
                            nc.tensor.matmul(ps0[:], xh_k, mh_sb[:, ko, 0:N_HALF], start=st, stop=False)
                            nc.tensor.matmul(ps1[:], xh_k, mh_sb[:, ko, N_HALF:F], start=st, stop=False)
                            nc.tensor.matmul(ps0[:], xh_k, ml_sb[:, ko, 0:N_HALF], start=False, stop=False)
                            nc.tensor.matmul(ps1[:], xh_k, ml_sb[:, ko, N_HALF:F], start=False, stop=False)
                            nc.tensor.matmul(ps0[:], xl_k, mh_sb[:, ko, 0:N_HALF], start=False, stop=sp)
                            nc.tensor.matmul(ps1[:], xl_k, mh_sb[:, ko, N_HALF:F], start=False, stop=sp)
                        nc.vector.tensor_copy(ot[:, 0:N_HALF], ps0[:])
                        nc.vector.tensor_copy(ot[:, N_HALF:F], ps1[:])
                        r0 = rc * R_CHUNK + rs * P
                        nc.gpsimd.dma_start(out[r0 : r0 + P, :], ot[:])
    nc.compile()
    _dedupe_ldweights(nc)
    return nc


def _build_nc(n_loops=1, mm_dtype=_FP32):
    """Plain single-dtype variant (fp32 reference path / experiments)."""
    nc = bacc.Bacc(None, target_bir_lowering=False)
    xT = nc.dram_tensor("xT", [F, RPC], mm_dtype, kind="ExternalInput")
    mm = nc.dram_tensor("mm", [F, F], mm_dtype, kind="ExternalInput")
    out = nc.dram_tensor("out", [RPC, F], _FP32, kind="ExternalOutput")

    with tile.TileContext(nc) as tc:
        with (
            tc.tile_pool(name="mpool", bufs=1) as mpool,
            tc.tile_pool(name="xpool", bufs=3) as xpool,
            tc.tile_pool(name="opool", bufs=3) as opool,
            tc.tile_pool(name="pspool", bufs=2, space="PSUM") as pspool,
        ):
            m_sb = mpool.tile([P, KO, F], mm_dtype)
            nc.sync.dma_start(m_sb[:], mm.rearrange("(ko p) n -> p ko n", p=P))

            xT3 = xT.rearrange("(ko p) r -> p ko r", p=P)
            for _it in range(n_loops):
                for rc in range(RPC // R_CHUNK):  # 16 panels
                    xt = xpool.tile([P, KO, R_CHUNK], mm_dtype)
                    nc.sync.dma_start(
                        xt[:], xT3[:, :, rc * R_CHUNK : (rc + 1) * R_CHUNK]
                    )
                    for rs in range(R_CHUNK // P):  # 4 row-tiles per panel
                        ot = opool.tile([P, F], _FP32)
                        for half in range(2):
                            ps = pspool.tile([P, N_HALF], _FP32, tag=f"ps{half}")
                            n0 = half * N_HALF
                            for ko in range(KO):
                                nc.tensor.matmul(
                                    ps[:],
                                    xt[:, ko, rs * P : (rs + 1) * P],
                                    m_sb[:, ko, n0 : n0 + N_HALF],
                                    start=(ko == 0),
                                    stop=(ko == KO - 1),
                                )
                            nc.vector.tensor_copy(ot[:, n0 : n0 + N_HALF], ps[:])
                        r0 = rc * R_CHUNK + rs * P
                        nc.sync.dma_start(out[r0 : r0 + P, :], ot[:])
    nc.compile()
    return nc


_NC_CACHE = {}


def _get_nc(mode="fp16x2"):
    if mode not in _NC_CACHE:
        if mode == "fp16x2":
            _NC_CACHE[mode] = _build_nc_fp16x2()
        elif mode == "fp32":
            _NC_CACHE[mode] = _build_nc()
        else:
            raise ValueError(mode)
    return _NC_CACHE[mode]


def _mlp(z, W1, b1, W2, b2):
    # (N,1) -> (N,1) fp32, mirrors reference._map_mlp
    h = np.tanh(z @ W1 + b1)
    return h @ W2 + b2


def _host_att_and_penalty(DistM, W1, b1, W2, b2):
    f = DistM.shape[0]
    att = _mlp(DistM.reshape(-1, 1), W1, b1, W2, b2).reshape(f, f)
    att = att * (1.0 - np.eye(f, dtype=att.dtype))
    att = att / (att.sum(axis=0, keepdims=True, dtype=np.float32) + EPS)
    m = att + np.eye(f, dtype=att.dtype)  # fold residual: x + x@Att = x @ (I+Att)

    kpts = np.linspace(0.0, 1.0, NUM_KPTS, dtype=np.float32).reshape(-1, 1)
    kout = _mlp(kpts, W1, b1, W2, b2)[:, 0]
    diff = kout[1:] - kout[:-1]
    penalty = 0.5 * PENALTY_RATE * np.mean(np.abs(diff) + diff, dtype=np.float32)
    return np.ascontiguousarray(m, dtype=np.float32), np.float32(penalty)


def _split16(a):
    hi = a.astype(np.float16)
    lo = (a - hi.astype(np.float32)).astype(np.float16)
    return hi, lo


def kernel(x, DistM, W1, b1, W2, b2, trace=False, trace_cores=None, mode="fp16x2"):
    global LAST_RESULTS
    x = np.asarray(x, dtype=np.float32)
    DistM = np.asarray(DistM, dtype=np.float32)
    W1 = np.asarray(W1, dtype=np.float32)
    b1 = np.asarray(b1, dtype=np.float32)
    W2 = np.asarray(W2, dtype=np.float32)
    b2 = np.asarray(b2, dtype=np.float32)

    m, penalty = _host_att_and_penalty(DistM, W1, b1, W2, b2)

    x2d = x.reshape(ROWS, F)
    if mode == "fp16x2":
        xT = np.ascontiguousarray(x2d.T)  # [F, ROWS]
        xh, xl = _split16(xT)
        mh, ml = _split16(m)
        in_maps = [
            {
                "xhT": xh[:, c * RPC : (c + 1) * RPC],
                "xlT": xl[:, c * RPC : (c + 1) * RPC],
                "mh": mh,
                "ml": ml,
            }
            for c in range(N_CORES)
        ]
    else:
        in_maps = [
            {"xT": x2d[c * RPC : (c + 1) * RPC].T, "mm": m} for c in range(N_CORES)
        ]

    kwargs = {}
    if trace:
        kwargs["trace"] = True
        if trace_cores is not None:
            kwargs["trace_cores"] = trace_cores

    try:
        res = run_bass_kernel_spmd(
            _get_nc(mode), in_maps, core_ids=list(range(N_CORES)), **kwargs
        )
    except ModuleNotFoundError:
        # Trace requested (possibly via BASS_TRACE env) but the axon NTFF
        # profile hook isn't available in this container — run untraced.
        import os

        os.environ["BASS_NEVER_TRACE"] = "1"
        res = run_bass_kernel_spmd(
            _get_nc(mode), in_maps, core_ids=list(range(N_CORES)), **kwargs
        )
    LAST_RESULTS = res

    out = np.concatenate(
        [res.results[c]["out"] for c in range(N_CORES)], axis=0
    ).reshape(B, S, F)
    return out, penalty


# revision 11
# speedup vs baseline: 9.3538x; 1.2237x over previous
"""Bass/Trainium2 kernel for nn_D2A_12086037971054 (sparse_attention).

Reference computation:
    Att[i,j] = MLP(DistM[i,j])           (per-scalar 1->64(tanh)->1 MLP)
    Att     *= (1 - I)                   (off-diagonal mask)
    Att     /= (Att.sum(axis=0) + EPS)   (column normalization)
    out      = x + x @ Att               ([16,4096,1024] @ [1024,1024])
    penalty  = scalar from 32 keypoints through the same MLP

Strategy (8 NeuronCores, data-parallel over batch per the sharding hint):
  - Att is tiny ([1024,1024], ~0.1% of total FLOPs) -> computed on host in
    fp32, residual folded in as M = I + Att so the device does one dense
    matmul out = x @ M (no separate residual add / second read of x).
  - x is viewed as [65536, 1024], split into 8 row-shards of 8192 rows, each
    shard transposed on host so the contraction dim lands on SBUF partitions
    with fully contiguous DMA.
  - Precision trick: fp32 matmul on the PE costs 4 cycles/row (two half-rate
    passes). Instead we split each operand into fp16 hi + fp16 lo
    (11+11 mantissa bits) and accumulate three fp16 matmuls in PSUM fp32:
        x@M ~= xh@Mh + xh@Ml + xl@Mh        (dropped xl@Ml ~ 2^-22 rel)
    3 cycles/row, HW-measured accuracy identical to the PE's native fp32
    path (~2e-7 max rel err), and the same DMA bytes as fp32.
  - penalty depends only on the MLP weights; computed host-side (replicated).
"""

import numpy as np

import concourse.mybir as mybir
from concourse import bacc, tile
from concourse.bass_utils import run_bass_kernel_spmd

N_CORES = 8
B, S, F, H = 16, 4096, 1024, 64
ROWS = B * S  # 65536
RPC = ROWS // N_CORES  # rows per core: 8192
PENALTY_RATE = 0.1
NUM_KPTS = 32
EPS = 1e-7

P = 128
KO = F // P  # 8 contraction tiles
R_CHUNK = 1024  # rows of x streamed per DMA panel
N_HALF = 512  # PSUM bank width (fp32)

_FP32 = mybir.dt.float32
_FP16 = mybir.dt.float16

LAST_RESULTS = None  # BassKernelResults of the most recent run (for test.py)


def _dedupe_ldweights(nc):
    """Drop PE InstLdweights that reload the exact weights AP already loaded
    by the immediately preceding (PE-stream) InstLdweights.

    tile_legalize splits every matmul into Ldweights + Matmult(ldweights=False);
    consecutive matmuls that share a stationary tile then carry redundant
    loads, which cost unhidden PE issue time at short (N=512 fp16, ~213ns)
    matmuls. Only sync-free loads are removed, so semaphore accounting is
    untouched; the PE weight state persists across matmuls, making the drop
    semantics-preserving."""
    import json as _json

    removed = 0
    for func in nc.m.functions:
        for block in func.blocks:
            new = []
            prev_sig = None
            for inst in block.instructions:
                if inst.engine == mybir.EngineType.PE:
                    if isinstance(inst, mybir.InstLdweights):
                        sig = str(inst.ins[0])
                        d = _json.loads(mybir.instruction_to_pretty_json_string(inst))
                        si = d.get("sync_info") or {}
                        clean = not si.get("on_wait") and not si.get("on_update")
                        if clean and sig == prev_sig:
                            removed += 1
                            continue
                        prev_sig = sig
                    elif isinstance(inst, mybir.InstMatmult):
                        if getattr(inst, "ldweights", None) is not False:
                            prev_sig = None  # self-loading matmul changes state
                    else:
                        prev_sig = None  # any other PE inst: be conservative
                new.append(inst)
            block.instructions = new
    return removed


def _build_nc_fp16x2(n_loops=1):
    """out = xh.T@Mh + xh.T@Ml + xl.T@Mh, all fp16 operands, fp32 PSUM.

    Matmuls are ordered so each stationary tile (xh_k / xl_k) serves the
    two PSUM half-banks back-to-back, letting _dedupe_ldweights drop 2/3 of
    the weight loads.

    n_loops > 1 repeats the whole compute body inside the NEFF; used only
    for differential HW timing ((t_N - t_1) / (N - 1))."""
    nc = bacc.Bacc(None, target_bir_lowering=False)
    xhT = nc.dram_tensor("xhT", [F, RPC], _FP16, kind="ExternalInput")
    xlT = nc.dram_tensor("xlT", [F, RPC], _FP16, kind="ExternalInput")
    mh = nc.dram_tensor("mh", [F, F], _FP16, kind="ExternalInput")
    ml = nc.dram_tensor("ml", [F, F], _FP16, kind="ExternalInput")
    out = nc.dram_tensor("out", [RPC, F], _FP32, kind="ExternalOutput")

    with tile.TileContext(nc) as tc:
        with (
            tc.tile_pool(name="mpool", bufs=1) as mpool,
            tc.tile_pool(name="xpool", bufs=3) as xpool,
            tc.tile_pool(name="opool", bufs=4) as opool,
            tc.tile_pool(name="pspool", bufs=2, space="PSUM") as pspool,
        ):
            # M hi/lo resident in SBUF: [128, 8, 1024] fp16 each.
            # DMA routing: x-hi + M-hi on the SP HWDGE ring, x-lo + M-lo on
            # the ACT HWDGE ring, output stores on the gpsimd SWDGE path —
            # three independent queues so panel loads never sit behind
            # output stores in a FIFO (measured ~100us of exposed stall when
            # everything shared the SP ring).
            mh_sb = mpool.tile([P, KO, F], _FP16, name="mh_sb")
            ml_sb = mpool.tile([P, KO, F], _FP16, name="ml_sb")
            nc.sync.dma_start(mh_sb[:], mh.rearrange("(ko p) n -> p ko n", p=P))
            nc.scalar.dma_start(ml_sb[:], ml.rearrange("(ko p) n -> p ko n", p=P))

            xh3 = xhT.rearrange("(ko p) r -> p ko r", p=P)
            xl3 = xlT.rearrange("(ko p) r -> p ko r", p=P)
            for _it in range(n_loops):
                for rc in range(RPC // R_CHUNK):  # 16 panels
                    xh_t = xpool.tile([P, KO, R_CHUNK], _FP16, name="xh_t")
                    xl_t = xpool.tile([P, KO, R_CHUNK], _FP16, name="xl_t")
                    rsl = slice(rc * R_CHUNK, (rc + 1) * R_CHUNK)
                    nc.sync.dma_start(xh_t[:], xh3[:, :, rsl])
                    nc.scalar.dma_start(xl_t[:], xl3[:, :, rsl])
                    for rs in range(R_CHUNK // P):  # 4 row-tiles per panel
                        ot = opool.tile([P, F], _FP32, name="ot")
                        ps0 = pspool.tile([P, N_HALF], _FP32, tag="ps0")
                        ps1 = pspool.tile([P, N_HALF], _FP32, tag="ps1")
                        for ko in range(KO):
                            xh_k = xh_t[:, ko, rs * P : (rs + 1) * P]
                            xl_k = xl_t[:, ko, rs * P : (rs + 1) * P]
                            st = ko == 0
                            sp = ko == KO - 1
                            # xh_k stationary: 4 matmuls; xl_k: 2 matmuls
                            nc.tensor.matmul(ps0[:], xh_k, mh_sb[:, ko, 0:N_HALF], start=st, stop=False)
                            nc.tensor.matmul(ps1[:], xh_k, mh_sb[:, ko, N_HALF:F], start=st, stop=False)
                            nc.tensor.matmul(ps0[:], xh_k, ml_sb[:, ko, 0:N_HALF], start=False, stop=False)
                            nc.tensor.matmul(ps1[:], xh_k, ml_sb[:, ko, N_HALF:F], start=False, stop=False)
                            nc.tensor.matmul(ps0[:], xl_k, mh_sb[:, ko, 0:N_HALF], start=False, stop=sp)
                            nc.tensor.matmul(ps1[:], xl_k, mh_sb[:, ko, N_HALF:F], start=False, stop=sp)
                        nc.vector.tensor_copy(ot[:, 0:N_HALF], ps0[:])
                        nc.vector.tensor_copy(ot[:, N_HALF:F], ps1[:])
                        r0 = rc * R_CHUNK + rs * P
                        nc.gpsimd.dma_start(out[r0 : r0 + P, :], ot[:])
    nc.compile()
    _dedupe_ldweights(nc)
    return nc


def _build_nc(n_loops=1, mm_dtype=_FP32):
    """Plain single-dtype variant (fp32 reference path / experiments)."""
    nc = bacc.Bacc(None, target_bir_lowering=False)
    xT = nc.dram_tensor("xT", [F, RPC], mm_dtype, kind="ExternalInput")
    mm = nc.dram_tensor("mm", [F, F], mm_dtype, kind="ExternalInput")
    out = nc.dram_tensor("out", [RPC, F], _FP32, kind="ExternalOutput")

    with tile.TileContext(nc) as tc:
        with (
            tc.tile_pool(name="mpool", bufs=1) as mpool,
            tc.tile_pool(name="xpool", bufs=3) as xpool,
            tc.tile_pool(name="opool", bufs=3) as opool,
            tc.tile_pool(name="pspool", bufs=2, space="PSUM") as pspool,
        ):
            m_sb = mpool.tile([P, KO, F], mm_dtype)
            nc.sync.dma_start(m_sb[:], mm.rearrange("(ko p) n -> p ko n", p=P))

            xT3 = xT.rearrange("(ko p) r -> p ko r", p=P)
            for _it in range(n_loops):
                for rc in range(RPC // R_CHUNK):  # 16 panels
                    xt = xpool.tile([P, KO, R_CHUNK], mm_dtype)
                    nc.sync.dma_start(
                        xt[:], xT3[:, :, rc * R_CHUNK : (rc + 1) * R_CHUNK]
                    )
                    for rs in range(R_CHUNK // P):  # 4 row-tiles per panel
                        ot = opool.tile([P, F], _FP32)
                        for half in range(2):
                            ps = pspool.tile([P, N_HALF], _FP32, tag=f"ps{half}")
                            n0 = half * N_HALF
                            for ko in range(KO):
                                nc.tensor.matmul(
                                    ps[:],
                                    xt[:, ko, rs * P : (rs + 1) * P],
                                    m_sb[:, ko, n0 : n0 + N_HALF],
                                    start=(ko == 0),
                                    stop=(ko == KO - 1),
                                )
                            nc.vector.tensor_copy(ot[:, n0 : n0 + N_HALF], ps[:])
                        r0 = rc * R_CHUNK + rs * P
                        nc.sync.dma_start(out[r0 : r0 + P, :], ot[:])
    nc.compile()
    return nc


_NC_CACHE = {}


def _get_nc(mode="fp16x2"):
    if mode not in _NC_CACHE:
        if mode == "fp16x2":
            _NC_CACHE[mode] = _build_nc_fp16x2()
        elif mode == "fp32":
            _NC_CACHE[mode] = _build_nc()
        else:
            raise ValueError(mode)
    return _NC_CACHE[mode]


def _mlp(z, W1, b1, W2, b2):
    # (N,1) -> (N,1) fp32, mirrors reference._map_mlp
    h = np.tanh(z @ W1 + b1)
    return h @ W2 + b2


def _host_att_and_penalty(DistM, W1, b1, W2, b2):
    f = DistM.shape[0]
    att = _mlp(DistM.reshape(-1, 1), W1, b1, W2, b2).reshape(f, f)
    att = att * (1.0 - np.eye(f, dtype=att.dtype))
    att = att / (att.sum(axis=0, keepdims=True, dtype=np.float32) + EPS)
    m = att + np.eye(f, dtype=att.dtype)  # fold residual: x + x@Att = x @ (I+Att)

    kpts = np.linspace(0.0, 1.0, NUM_KPTS, dtype=np.float32).reshape(-1, 1)
    kout = _mlp(kpts, W1, b1, W2, b2)[:, 0]
    diff = kout[1:] - kout[:-1]
    penalty = 0.5 * PENALTY_RATE * np.mean(np.abs(diff) + diff, dtype=np.float32)
    return np.ascontiguousarray(m, dtype=np.float32), np.float32(penalty)


def _split16(a):
    hi = a.astype(np.float16)
    lo = (a - hi.astype(np.float32)).astype(np.float16)
    return hi, lo


def kernel(x, DistM, W1, b1, W2, b2, trace=False, trace_cores=None, mode="fp16x2"):
    global LAST_RESULTS
    x = np.asarray(x, dtype=np.float32)
    DistM = np.asarray(DistM, dtype=np.float32)
    W1 = np.asarray(W1, dtype=np.float32)
    b1 = np.asarray(b1, dtype=np.float32)
    W2 = np.asarray(W2, dtype=np.float32)
    b2 = np.asarray(b2, dtype=np.float32)

    m, penalty = _host_att_and_penalty(DistM, W1, b1, W2, b2)

    x2d = x.reshape(ROWS, F)
    if mode == "fp16x2":
        xT = np.ascontiguousarray(x2d.T)  # [F, ROWS]
        xh, xl = _split16(xT)
        mh, ml = _split16(m)
        in_maps = [
            {
                "xhT": xh[:, c * RPC : (c + 1) * RPC],
                "xlT": xl[:, c * RPC : (c + 1) * RPC],
                "mh": mh,
                "ml": ml,
            }
            for c in range(N_CORES)
        ]
    else:
        in_maps = [
            {"xT": x2d[c * RPC : (c + 1) * RPC].T, "mm": m} for c in range(N_CORES)
        ]

    kwargs = {}
    if trace:
        kwargs["trace"] = True
        if trace_cores is not None:
            kwargs["trace_cores"] = trace_cores

    try:
        res = run_bass_kernel_spmd(
            _get_nc(mode), in_maps, core_ids=list(range(N_CORES)), **kwargs
        )
    except ModuleNotFoundError:
        # Trace requested (possibly via BASS_TRACE env) but the axon NTFF
        # profile hook isn't available in this container — run untraced.
        import os

        os.environ["BASS_NEVER_TRACE"] = "1"
        res = run_bass_kernel_spmd(
            _get_nc(mode), in_maps, core_ids=list(range(N_CORES)), **kwargs
        )
    LAST_RESULTS = res

    out = np.concatenate(
        [res.results[c]["out"] for c in range(N_CORES)], axis=0
    ).reshape(B, S, F)
    return out, penalty
